# revision 45
# baseline (speedup 1.0000x reference)
"""Trainium2 Bass kernel for the two-template sparse cross-modal attention module.

Sharding: data-parallel over batch B=32 across 8 NeuronCores (4 samples/core).
Each sample carries two modality streams (v, i) that must be co-resident
because search tokens attend to the template keys of BOTH modalities.

Per-core program (per sample s, streams st in {v, i}), all matmuls bf16 with
fp32 PSUM accumulation (inputs are cast to bf16 by the gpsimd DMAs, which
also halves input HBM traffic):
  1. QK^T projection in transposed layout (QKT[1536, 384] = qkv_w[0:1536] @
     x.T, stored bf16); V projection in natural layout with a ones column per
     head ([tok, 65]) so the AV matmul also accumulates the softmax
     denominator.
  2. Scores transposed (S.T[k, q] = K Q.T, contract Dh=64), exp on the ACT
     engine (no max-subtraction; logits are O(1)). Each score matmul writes a
     whole single-bank PSUM tile at offset 0: a start=True accumulation-group
     open at a non-zero PSUM bank offset faults on real hardware.
  3. AV in natural orientation: out[q, 65] = es.T @ V1 with es (the exp'ed
     score tile) as the stationary operand — each matmul streams only 65
     columns instead of 256/384, which is what the PE cost scales with. The
     six accumulation chains of a head-pair share one PSUM bank as ONE group
     (only the offset-0 chain opens it). Normalization = DVE batched
     reciprocal of the ones column + per-head scaled PSUM-drain copies into
     the natural-layout O tile (bf16), emitted in reverse so the bank is
     fully drained before the next group's first AV write (PSUM banks are
     single-ported; a PE write concurrent with a DVE read anywhere in the
     bank is fatal).
  4. O is transposed back to channel-major via PE transpose instructions
     (bf16 identity), then the output projection runs from the transposed
     tile; the bias is added by the DVE during the PSUM drain (no K=1 bias
     matmuls).

Scheduling: projection matmul blocks are held in a keyed filler queue and
interleaved between each head-pair's score and AV matmuls so the ACT engine's
exp latency (the attention-phase bottleneck) hides under PE work. Each window
force-emits just-in-time the P1/P2 blocks the next window's scores/AV read
and the deferred output-projection blocks whose o_nat buffers are about to be
recycled; surplus blocks ride across sample boundaries toward the
filler-starved last sample. Input DMAs are issued as batched multi-chunk
waves in exact consumption order, so the first projection matmul starts ~6us
into the program.

Cost-model numbers (TimelineSim, per core): 288us total, PE busy 265us
(vs 412us / 290us for the previous kernel).
"""

import numpy as np

for _p in ("/opt/trn_rl_repo", "/root/.axon_site/_ro/trn_rl_repo"):
    import os
    import sys

    if os.path.isdir(_p) and _p not in sys.path:
        sys.path.append(_p)

B = 32
N_CORES = 8
SAMPLES = 4  # per core
C = 768
NTOK = 384
H = 12
DH = 64
MT = 128  # template tokens
CCH = C // 128  # 6 contraction chunks
MCH = 12  # QK row chunks (1536/128)
TCH = NTOK // 128  # 3 token chunks
SCALE = DH ** (-0.5)

_PROG_CACHE = {}

# Filler block cost estimates (ns) for pacing the interleave.
_COST_P12_BLOCK = 1100.0  # 6 matmuls of N=384 + drain dispatch
_COST_T_BLOCK = 2400.0  # 6 transposes + 12 proj matmuls of N=384
_GROUP_FILL_NS = 2200.0  # target filler time between S(k) and AV(k)


def _build_program(mm_f32r=True, es_bf16=True, with_bias=True):
    import concourse.bass as bass  # noqa: F401
    import concourse.tile as tile
    from concourse import bacc, masks, mybir

    f32 = mybir.dt.float32
    f32r = mybir.dt.float32r
    bf16 = mybir.dt.bfloat16
    Act = mybir.ActivationFunctionType

    nc = bacc.Bacc(None, target_bir_lowering=False)
    _lp = nc.allow_low_precision(
        reason="fp32r/bf16 matmul inputs, fp32 PSUM accumulation"
    )
    _lp.__enter__()

    xt_d = nc.dram_tensor("xt", [SAMPLES, C, 2, NTOK], f32, kind="ExternalInput")
    qkvw_d = nc.dram_tensor("qkvwT", [C, 3 * C], f32, kind="ExternalInput")
    projw_d = nc.dram_tensor("projwT", [C, C], f32, kind="ExternalInput")
    bias_d = nc.dram_tensor("bias", [1, C], f32, kind="ExternalInput")
    y_d = nc.dram_tensor("y", [2 * SAMPLES, NTOK, C], f32, kind="ExternalOutput")

    with tile.TileContext(nc) as tc:
        with (
            tc.tile_pool(name="consts", bufs=1) as consts,
            tc.tile_pool(name="xtp", bufs=2) as xtp,
            tc.tile_pool(name="qktp", bufs=2) as qktp,
            tc.tile_pool(name="v1p", bufs=2) as v1p,
            tc.tile_pool(name="onp", bufs=6) as onp,
            tc.tile_pool(name="otp", bufs=1) as otp,
            tc.tile_pool(name="esap", bufs=4) as esap,
            tc.tile_pool(name="esbp", bufs=6) as esbp,
            tc.tile_pool(name="rlp", bufs=4) as rlp,
            tc.tile_pool(name="yp", bufs=3) as yp,
            tc.tile_pool(name="psp", bufs=4, space="PSUM") as psp,
            tc.tile_pool(name="pop", bufs=2, space="PSUM") as pop,
            tc.tile_pool(name="pap", bufs=2, space="PSUM") as pap,
        ):
            # ---- constant tiles ----
            qkvw_sb = consts.tile([128, CCH, 3 * C], bf16)
            projw_sb = consts.tile([128, CCH, C], bf16)
            bias_row = consts.tile([1, C], f32)
            bias_bc = consts.tile([128, C], f32)
            ident = consts.tile([128, 128], bf16)
            masks.make_identity(nc, ident)

            # ---- weight DMA waves, in consumption order ----
            # One batched DMA per wave (all 6 c-chunks at once via a
            # rearranged DRAM access pattern) — each gpsimd cast-DMA costs
            # ~1us of Pool prep time, so issue count matters for the startup.
            def qkvw_wave(m0, m1):
                nc.gpsimd.dma_start(
                    out=qkvw_sb[:, :, m0 * 128 : m1 * 128],
                    in_=qkvw_d[:, m0 * 128 : m1 * 128].rearrange(
                        "(c p) m -> p c m", p=128
                    ),
                )



            # ---- per-sample tiles (created lazily, in emission order) ----
            tiles = {}

            def xt_dma(s, st):
                xt = tiles[s][0]
                nc.gpsimd.dma_start(
                    out=xt[:, :, st, :],
                    in_=xt_d[s, :, st, :].rearrange("(c p) n -> p c n", p=128),
                )

            def make_sample_tiles(s, dma=True):
                xt = xtp.tile([128, CCH, 2, NTOK], bf16, tag="xt", name=f"xt_{s}")
                qkt = qktp.tile([128, MCH, 2, NTOK], bf16, tag="qkt", name=f"qkt_{s}")
                v1 = v1p.tile([128, TCH, 2, H, 65], bf16, tag="v1", name=f"v1_{s}")
                # ones column for the softmax denominator
                nc.vector.memset(v1[:, :, :, :, 64:65], 1.0)
                tiles[s] = (xt, qkt, v1)
                # one batched cast-DMA per stream so P1(st=0) can start after
                # half the transfer
                if dma:
                    xt_dma(s, 0)
                    xt_dma(s, 1)

            def p1_block(s, m, st):
                xt, qkt, _ = tiles[s]
                pq = pap.tile([128, NTOK], f32, tag="pa", name=f"pq_{s}_{m}_{st}")
                for c in range(CCH):
                    nc.tensor.matmul(
                        pq,
                        qkvw_sb[:, c, m * 128 : (m + 1) * 128],
                        xt[:, c, st, :],
                        start=(c == 0),
                        stop=(c == CCH - 1),
                    )
                nc.vector.tensor_copy(out=qkt[:, m, st, :], in_=pq)

            def p2_block(s, t, st, n):
                xt, _, v1 = tiles[s]
                pv = pap.tile([128, NTOK], f32, tag="pa", name=f"pv_{s}_{t}_{st}_{n}")
                for c in range(CCH):
                    nc.tensor.matmul(
                        pv,
                        xt[:, c, st, t * 128 : (t + 1) * 128],
                        qkvw_sb[:, c, 2 * C + n * NTOK : 2 * C + (n + 1) * NTOK],
                        start=(c == 0),
                        stop=(c == CCH - 1),
                    )
                nc.vector.tensor_copy(
                    out=v1[:, t, st, 6 * n : 6 * n + 6, 0:64],
                    in_=pv.rearrange("p (h d) -> p h d", h=6),
                )

            def t_block(s, st, t, o_nat, y_dst, fine_out=False):
                # transposes for output t-chunk + output projection + bias + DMA
                _, qkt, _ = tiles[s]
                y_sb = yp.tile([128, C], f32, tag="y", name=f"y_{s}_{st}_{t}")
                for g in range(2):
                    ptr = pap.tile([128, 3, 128], bf16, tag="pa", name=f"pt_{s}_{st}_{t}_{g}")
                    for j in range(3):
                        c = 3 * g + j
                        nc.tensor.transpose(
                            ptr[:, j, :], o_nat[:, c * 128 : (c + 1) * 128], ident
                        )
                    nc.vector.tensor_copy(
                        out=ot_sb[:, 3 * g : 3 * g + 3, st, t * 128 : (t + 1) * 128],
                        in_=ptr,
                    )
                for n2 in range(2):
                    py = pap.tile([128, NTOK], f32, tag="pa", name=f"py_{s}_{st}_{t}_{n2}")
                    for c in range(CCH):
                        nc.tensor.matmul(
                            py,
                            ot_sb[:, c, st, t * 128 : (t + 1) * 128],
                            projw_sb[:, c, n2 * NTOK : (n2 + 1) * NTOK],
                            start=(c == 0),
                            stop=(c == CCH - 1),
                        )
                    nparts = 1
                    w = NTOK // nparts
                    for q in range(nparts):
                        lo = n2 * NTOK + q * w
                        hi = lo + w
                        if with_bias:
                            nc.vector.tensor_add(
                                y_sb[:, lo:hi], py[:, q * w : (q + 1) * w],
                                bias_bc[:, lo:hi],
                            )
                        else:
                            nc.vector.tensor_copy(
                                out=y_sb[:, lo:hi], in_=py[:, q * w : (q + 1) * w]
                            )
                        nc.sync.dma_start(
                            out=y_dst[t * 128 : (t + 1) * 128, lo:hi],
                            in_=y_sb[:, lo:hi],
                        )

            ot_sb = consts.tile([128, CCH, 2, NTOK], bf16)

            def av_norm_body(s, st, hp, esA, esB, esC, esD, v1, o_nat_qc):
                # AV, natural orientation (N=65 per matmul). The whole po bank
                # is ONE accumulation group: only the very first matmul (i=0,
                # qc=0, bank offset 0) opens it with start=True; later matmuls
                # continue with start=False (fresh addresses overwrite via
                # has_written, revisited addresses accumulate).
                po = pop.tile([128, 2, TCH, 65], f32, tag="po", name=f"po_{s}_{st}_{hp}")
                for i in range(2):
                    h = 2 * hp + i
                    nc.tensor.matmul(
                        po[:, i, 0, :],
                        esA[:, i, 0:128],
                        v1[:, 0, st, h, :],
                        start=(i == 0),
                        stop=False,
                    )
                    for qc in (1, 2):
                        q0 = qc * 128
                        dst = po[:, i, qc, :]
                        nc.tensor.matmul(
                            dst, esA[:, i, q0 : q0 + 128], v1[:, 0, st, h, :],
                            start=False, stop=False,
                        )
                        nc.tensor.matmul(
                            dst, esB[:, i, q0 - 128 : q0], v1[:, 0, 1 - st, h, :],
                            start=False, stop=False,
                        )
                        nc.tensor.matmul(
                            dst, esC[:, i, q0 - 128 : q0], v1[:, 1, st, h, :],
                            start=False, stop=False,
                        )
                        nc.tensor.matmul(
                            dst, esD[:, i, q0 - 128 : q0], v1[:, 2, st, h, :],
                            start=False, stop=(i == 1 and qc == 2),
                        )
                # normalization: batched reciprocal + scaled drain, in reverse
                # so the last DVE read of the bank covers the addresses the
                # next group's first AV matmul writes (PSUM bank-collision
                # safety via the DVE FIFO).
                rl = rlp.tile([128, 2, TCH], f32, tag="rl", name=f"rl_{s}_{st}_{hp}")
                nc.vector.reciprocal(out=rl, in_=po[:, :, :, 64])
                for i in (1, 0):
                    h = 2 * hp + i
                    for qc in (2, 1, 0):
                        nc.vector.tensor_scalar_mul(
                            o_nat_qc[qc][:, h * 64 : (h + 1) * 64],
                            po[:, i, qc, 0:64],
                            rl[:, i, qc : qc + 1],
                        )

            # ---- filler queue with just-in-time forcing ----
            # Entries are [cost, key, fn, done]. Windows pop FIFO up to a
            # budget; `force(key)` emits a specific pending block immediately
            # (used to guarantee the qkt/v1 chunks a window reads were
            # produced in an earlier window). Surplus blocks ride forward
            # across sample boundaries toward the filler-starved last sample.
            fill_q = []
            pending = {}

            def push(cost, key, fn):
                e = [cost, key, fn, False]
                fill_q.append(e)
                if key is not None:
                    pending[key] = e

            def emit_entry(e):
                if e[3]:
                    return 0.0
                e[3] = True
                e[2]()
                if e[1] is not None:
                    pending.pop(e[1], None)
                return e[0]

            def force(key):
                e = pending.get(key)
                return emit_entry(e) if e is not None else 0.0

            def drain_fillers(budget):
                while fill_q and budget > 0.0:
                    e = fill_q[0]
                    if e[3]:
                        fill_q.pop(0)
                        continue
                    budget -= emit_entry(e)
                    fill_q.pop(0)

            def flush_all():
                for e in list(fill_q):
                    emit_entry(e)
                fill_q.clear()

            def queue_p12(s):
                for m in range(MCH):
                    for st in range(2):
                        push(
                            _COST_P12_BLOCK,
                            ("p1", s, m, st),
                            lambda s=s, m=m, st=st: p1_block(s, m, st),
                        )
                for t in range(TCH):
                    for st in range(2):
                        for n in range(2):
                            push(
                                _COST_P12_BLOCK,
                                ("p2", s, t, st, n),
                                lambda s=s, t=t, st=st, n=n: p2_block(s, t, st, n),
                            )

            held_t = []

            def queue_proj(s, st, o_nat_qc):
                for t in range(TCH):
                    fine = False
                    fn = lambda s=s, st=st, t=t, o=o_nat_qc[t], f=fine: t_block(
                        s, st, t, o, y_d[2 * s + st], fine_out=f
                    )
                    if s == SAMPLES - 1 and st == 0:
                        # reserve for the filler-starved last windows
                        held_t.append(fn)
                    else:
                        push(_COST_T_BLOCK, ("t", s, st, t), fn)

            def s_needs(s, st, hp):
                return (
                    ("p1", s, hp, st),
                    ("p1", s, 6 + hp, st),
                    ("p1", s, 6 + hp, 1 - st),
                )

            def av_needs(s, st, hp):
                n = hp // 3
                return (
                    ("p2", s, 0, st, n),
                    ("p2", s, 1, st, n),
                    ("p2", s, 2, st, n),
                    ("p2", s, 0, 1 - st, n),
                )

            # ---- sample 0 projections run inline ----
            # st-outer so P1(st=0) starts as soon as the st=0 half of xt and
            # the first weight wave land; weight waves interleave in
            # consumption order.
            make_sample_tiles(0, dma=False)
            xt_dma(0, 0)
            qkvw_wave(0, 1)
            qkvw_wave(1, 3)
            qkvw_wave(3, 6)
            qkvw_wave(6, 9)
            qkvw_wave(9, 12)
            xt_dma(0, 1)
            qkvw_wave(12, 15)
            qkvw_wave(15, 18)
            nc.gpsimd.dma_start(
                out=projw_sb,
                in_=projw_d.rearrange("(c p) m -> p c m", p=128),
            )
            nc.sync.dma_start(out=bias_row, in_=bias_d[:, :])
            nc.gpsimd.partition_broadcast(bias_bc, bias_row)
            for st in range(2):
                for m in range(MCH):
                    p1_block(0, m, st)
            make_sample_tiles(1)
            for t in range(TCH):
                for st in range(2):
                    for n in range(2):
                        p2_block(0, t, st, n)

            # ---- main loop ----
            for s in range(SAMPLES):
                _, qkt, v1 = tiles[s]
                if s + 1 < SAMPLES:
                    queue_p12(s + 1)
                windows = [(st, hp) for st in range(2) for hp in range(6)]
                for st in range(2):
                    o_nat_qc = [
                        onp.tile([128, C], bf16, tag="on", name=f"on_{s}_{st}_{qc}")
                        for qc in range(TCH)
                    ]
                    for hp in range(6):
                        # backstop: anything this window reads must exist now
                        forced = 0.0
                        for k in s_needs(s, st, hp) + av_needs(s, st, hp):
                            forced += force(k)
                        # ---- scores (S.T = K Q.T), bf16, transposed layout ----
                        # Every matmul writes a whole single-bank tile at
                        # offset 0 (matmuls writing at a non-zero PSUM bank
                        # offset fault on hardware). 8 tiles rotate through 4
                        # banks; the exp of each tile is emitted right after
                        # its matmul so the bank frees quickly.
                        esA = esap.tile([128, 2, NTOK], bf16, tag="esa", name=f"esA_{s}_{st}_{hp}")
                        esB = esbp.tile([128, 2, 256], bf16, tag="esb", name=f"esB_{s}_{st}_{hp}")
                        esC = esbp.tile([128, 2, 256], bf16, tag="esb", name=f"esC_{s}_{st}_{hp}")
                        esD = esbp.tile([128, 2, 256], bf16, tag="esb", name=f"esD_{s}_{st}_{hp}")
                        for i in range(2):
                            ro = 64 * i
                            qT = qkt[ro : ro + 64, hp, st, :]
                            qTs = qkt[ro : ro + 64, hp, st, MT:]
                            kT = qkt[ro : ro + 64, 6 + hp, st, :]
                            kTo = qkt[ro : ro + 64, 6 + hp, 1 - st, :]
                            for letter, lk, rq, nq, es in (
                                ("A", kT[:, 0:MT], qT, NTOK, esA),
                                ("B", kTo[:, 0:MT], qTs, 256, esB),
                                ("C", kT[:, MT : MT + 128], qTs, 256, esC),
                                ("D", kT[:, MT + 128 : MT + 256], qTs, 256, esD),
                            ):
                                ps = psp.tile(
                                    [128, nq], f32, tag="ps",
                                    name=f"ps{letter}_{s}_{st}_{hp}_{i}",
                                )
                                nc.tensor.matmul(ps, lk, rq, start=True, stop=True)
                                nc.scalar.activation(
                                    es[:, i, :], ps, Act.Exp, scale=SCALE
                                )

                        # ---- fillers: hide exp latency under projection work ----
                        # lookahead: produce the NEXT window's inputs here so
                        # the next window's scores/AV never wait on a fresh
                        # qkt/v1 write
                        wi = windows.index((st, hp))
                        for la in (1, 2):
                            if wi + la < len(windows):
                                nst, nhp = windows[wi + la]
                                for k in s_needs(s, nst, nhp) + av_needs(s, nst, nhp):
                                    forced += force(k)
                        if s + 1 < SAMPLES:
                            # pre-produce the next sample's first window late in
                            # this sample
                            if (st, hp) == (1, 4):
                                for k in av_needs(s + 1, 0, 0):
                                    forced += force(k)
                            if (st, hp) == (1, 5):
                                for k in s_needs(s + 1, 0, 0) + av_needs(s + 1, 0, 0):
                                    forced += force(k)
                        # deadline for deferred output-projection blocks: the
                        # o_nat buffers they read are overwritten one stream
                        # later, so (s-1, st1) must run during (s, st0) and
                        # (s, st0) during (s, st1)
                        if hp >= 3:
                            if st == 1 and s == SAMPLES - 1:
                                if held_t:
                                    held_t.pop(0)()
                                    forced += _COST_T_BLOCK
                            else:
                                tk = (
                                    ("t", s - 1, 1, hp - 3)
                                    if st == 0
                                    else ("t", s, 0, hp - 3)
                                )
                                forced += force(tk)
                        drain_fillers(max(0.0, _GROUP_FILL_NS - forced))

                        av_norm_body(s, st, hp, esA, esB, esC, esD, v1, o_nat_qc)


                    # ---- end of stream: defer this stream's projections ----
                    queue_proj(s, st, o_nat_qc)
                    if s == 0 and st == 0:
                        # xt for sample 1 was created before p2; issue s+1 early
                        pass

                # ---- end of sample ----
                if s + 1 < SAMPLES:
                    if s + 2 < SAMPLES:
                        make_sample_tiles(s + 2)
                else:
                    for fn in held_t:
                        fn()
                    held_t.clear()
                    flush_all()

    _lp.__exit__(None, None, None)
    nc.compile()
    return nc


def _get_program(mm_f32r=True, es_bf16=True, with_bias=True):
    key = (bool(with_bias),)
    if key not in _PROG_CACHE:
        _PROG_CACHE[key] = _build_program(with_bias=bool(with_bias))
    return _PROG_CACHE[key]


def _prep_in_maps(x_v, x_i, qkv_w, proj_w, proj_b):
    qkvwT = np.ascontiguousarray(np.asarray(qkv_w).T.astype(np.float32))
    projwT = np.ascontiguousarray(np.asarray(proj_w).T.astype(np.float32))
    bias = np.ascontiguousarray(np.asarray(proj_b).astype(np.float32).reshape(1, C))
    in_maps = []
    for core in range(N_CORES):
        sl = slice(core * SAMPLES, (core + 1) * SAMPLES)
        xs = np.empty((SAMPLES, C, 2, NTOK), np.float32)
        xs[:, :, 0, :] = np.asarray(x_v[sl]).transpose(0, 2, 1)
        xs[:, :, 1, :] = np.asarray(x_i[sl]).transpose(0, 2, 1)
        in_maps.append(
            {
                "xt": np.ascontiguousarray(xs),
                "qkvwT": qkvwT,
                "projwT": projwT,
                "bias": bias,
            }
        )
    return in_maps


def kernel(x_v, x_i, qkv_w, proj_w, proj_b, t_h, t_w, s_h, s_w, num_heads):
    from concourse.bass_utils import run_bass_kernel_spmd

    x_v = np.asarray(x_v, np.float32)
    x_i = np.asarray(x_i, np.float32)
    nc = _get_program(with_bias=bool(np.any(np.asarray(proj_b))))
    in_maps = _prep_in_maps(x_v, x_i, qkv_w, proj_w, proj_b)
    res = run_bass_kernel_spmd(nc, in_maps, list(range(N_CORES)))
    out_v = np.empty((B, NTOK, C), np.float32)
    out_i = np.empty((B, NTOK, C), np.float32)
    for core in range(N_CORES):
        y = res.results[core]["y"]
        sl = slice(core * SAMPLES, (core + 1) * SAMPLES)
        out_v[sl] = y[0::2]
        out_i[sl] = y[1::2]
    return out_v, out_i
